# revision 18
# baseline (speedup 1.0000x reference)
"""MixHop layer (hop0 + A@h1 + A^2@h2) on 8 trn2 NeuronCores.

Strategy (v2): 1D node partition across 8 cores with host-side balancing
relabel. Linearity trick: S = A@x is computed ONCE per core (scatter
matmuls with host-PRE-GATHERED x rows streamed sequentially -> no on-device
gather in pass A); then y1 = S@W1 + rowsum*b1 and g = S@W2 + rowsum*b2 on
TensorE. One AllGather of g (split into two window-halves for overlap),
then pass B scatters y2 = A@g using SWDGE dma_gather of g rows (256B) +
one-hot scatter matmuls. Outputs written bf16 (partly transposed); host
fixes layout. The only collective is the g AllGather; the only on-device
gather pass is pass B.
"""
import heapq
import os
import sys

for p in ("/opt/trn_rl_repo", "/root/.axon_site/_ro/trn_rl_repo"):
    if os.path.isdir(p) and p not in sys.path:
        sys.path.append(p)

import numpy as np
import ml_dtypes

N = 50000
E = 600000
C = 128
CORES = 8
NW = 50                   # windows per core
RPC = NW * 128            # 6400 rows per core (padded)
NP = RPC * CORES          # 51200
NWH = NW // 2             # 25 windows per half
HROWS = NWH * 128 * CORES  # 25600 rows per AllGather half (int16-safe)
QW = [(0, 13), (13, 25), (25, 38), (38, 50)]   # AllGather quarters
QROWS = [(b - a) * 128 for a, b in QW]         # per-core rows per quarter
NQ = 4                    # SWDGE queues
PTDVE = float(os.environ.get("KM_PTDVE", "0.70"))


# supergroup ramps (must have a boundary exactly at NWH for the AG split)
def _ramp(sizes):
    out, w = [], 0
    for s in sizes:
        out.append((w, min(NW, w + s)))
        w += s
        if w >= NW:
            break
    return out

SG_A = _ramp([2, 3, 5, 3, 5, 5, 2, 5, 5, 3, 5, 5, 2])
SG_B = _ramp([2, 3, 5, 5, 5, 5, 5, 5, 5, 5, 5])
for b in (13, 25, 38):
    assert any(w1 == b for _, w1 in SG_A), (b, SG_A)

TRACE = False
STAGES = int(os.environ.get("KM_STAGES", "5"))
LAG = int(os.environ.get("KM_LAG", "2"))
_CACHE = {}


def _balance_perm(edge_row):
    """Assign nodes to (core, window) slots balancing per-slot edge counts.
    Returns relabel[old_row] = new_row = core*RPC + window*128 + k."""
    deg = np.bincount(edge_row, minlength=N).astype(np.int64)
    order = np.argsort(-deg, kind="stable")
    nslots = CORES * NW
    loads = [(0, s) for s in range(nslots)]
    heapq.heapify(loads)
    space = np.full(nslots, 128, np.int64)
    new_of_old = np.empty(NP, np.int64)
    for r in order:
        while True:
            load, s = heapq.heappop(loads)
            if space[s] > 0:
                break
        k = 128 - space[s]
        space[s] -= 1
        new_of_old[r] = s * 128 + k
        if space[s] > 0:
            heapq.heappush(loads, (load + deg[r], s))
    rem = []
    for s in range(nslots):
        for k in range(128 - space[s], 128):
            rem.append(s * 128 + k)
    new_of_old[N:] = rem
    return new_of_old


def _build_plan(edge_row, edge_col, edge_val):
    relabel = _balance_perm(edge_row)
    er = relabel[edge_row]
    ec = relabel[edge_col]

    core = er // RPC
    w = (er % RPC) // 128
    off = (er % 128).astype(np.int64)

    # ---- pass A: chunks grouped by (core, window); x rows pre-gathered ----
    gidA = core * NW + w
    cntA = np.bincount(gidA, minlength=CORES * NW).reshape(CORES, NW)
    BwA = np.maximum(1, (cntA.max(axis=0) + 127) // 128)      # [NW]
    cstartA = np.zeros(NW + 1, np.int64)
    np.cumsum(BwA, out=cstartA[1:])
    T_A = int(cstartA[NW])

    orderA = np.argsort(gidA, kind="stable")
    gsA = np.zeros(CORES * NW + 1, np.int64)
    np.cumsum(cntA.reshape(-1), out=gsA[1:])
    rankA = np.arange(E, dtype=np.int64) - gsA[gidA[orderA]]
    posA = cstartA[w[orderA]] * 128 + rankA
    flatA = core[orderA] * (T_A * 128) + posA

    colA = np.zeros(CORES * T_A * 128, np.int64)   # source node per slot
    colA[flatA] = ec[orderA]
    colA = colA.reshape(CORES, T_A * 128)

    ptA = np.zeros((CORES * T_A * 128, 128), ml_dtypes.bfloat16)
    ptA[flatA, off[orderA]] = edge_val[orderA].astype(ml_dtypes.bfloat16)
    ptA = ptA.reshape(CORES, T_A, 128, 128).transpose(0, 2, 1, 3)
    ptA = np.ascontiguousarray(ptA.reshape(CORES, 128, T_A * 128))

    # ---- pass B: chunks grouped by (core, window, src-half) ----
    hcore = ec // RPC
    hw = (ec % RPC) // 128
    half = (hw >= NWH).astype(np.int64)
    flrow = hcore * (NWH * 128) + (hw - NWH * half) * 128 + (ec % 128)

    gidB = (core * NW + w) * 2 + half
    cntB = np.bincount(gidB, minlength=CORES * NW * 2).reshape(CORES, NW, 2)
    BwB = np.maximum(1, (cntB.max(axis=0) + 127) // 128)      # [NW, 2]

    cstartB = np.zeros((NW, 2), np.int64)
    callsB = []          # per (sg, half): dict(c0, nch)
    cpos = 0
    for (w0, w1) in SG_B:
        for h in (0, 1):
            nch = int(BwB[w0:w1, h].sum())
            for wi in range(w0, w1):
                cstartB[wi, h] = cpos
                cpos += int(BwB[wi, h])
            callsB.append(dict(h=h, w0=w0, w1=w1, c0=cpos - nch, nch=nch))
    T_B = cpos

    orderB = np.argsort(gidB, kind="stable")
    gsB = np.zeros(CORES * NW * 2 + 1, np.int64)
    np.cumsum(cntB.reshape(-1), out=gsB[1:])
    rankB = np.arange(E, dtype=np.int64) - gsB[gidB[orderB]]
    posB = cstartB[w[orderB], half[orderB]] * 128 + rankB
    flatB = core[orderB] * (T_B * 128) + posB

    idxB = np.zeros(CORES * T_B * 128, np.int16)
    idxB[flatB] = flrow[orderB].astype(np.int16)
    idxB = idxB.reshape(CORES, T_B, 128)

    ptB = np.zeros((CORES * T_B * 128, 128), ml_dtypes.bfloat16)
    ptB[flatB, off[orderB]] = edge_val[orderB].astype(ml_dtypes.bfloat16)
    ptB = ptB.reshape(CORES, T_B, 128, 128).transpose(0, 2, 1, 3)
    ptB = np.ascontiguousarray(ptB.reshape(CORES, 128, T_B * 128))

    seg = idxB.reshape(CORES, T_B * 128 // 16, 16)
    wrapped16 = seg.transpose(0, 2, 1)
    gix = np.ascontiguousarray(np.tile(wrapped16, (1, 8, 1)))

    rowsum = np.bincount(er, weights=edge_val.astype(np.float64),
                         minlength=NP).astype(np.float32)

    offa = np.zeros(CORES * T_A * 128, np.float32)
    vala = np.zeros(CORES * T_A * 128, np.float32)
    offa[flatA] = off[orderA].astype(np.float32)
    vala[flatA] = edge_val[orderA]
    offa = np.ascontiguousarray(offa.reshape(CORES, T_A, 128).transpose(0, 2, 1))
    vala = np.ascontiguousarray(vala.reshape(CORES, T_A, 128).transpose(0, 2, 1))

    return dict(relabel=relabel, BwA=BwA, cstartA=cstartA, T_A=T_A,
                colA=colA, ptA=ptA, BwB=BwB, cstartB=cstartB,
                callsB=callsB, T_B=T_B, ptB=ptB, gix=gix, rowsum=rowsum,
                offa=offa, vala=vala)


def _build_program(plan):
    import concourse.bass as bass
    import concourse.bacc as bacc
    import concourse.mybir as mybir
    import concourse.tile as tile

    dt = mybir.dt
    BwA, cstartA, T_A = plan["BwA"], plan["cstartA"], plan["T_A"]
    BwB, cstartB, callsB, T_B = (plan["BwB"], plan["cstartB"],
                                 plan["callsB"], plan["T_B"])

    nc = bacc.Bacc("TRN2", target_bir_lowering=False, debug=False,
                   num_devices=CORES, num_swdge_queues=NQ)

    xT_d = nc.dram_tensor("xT", [128, RPC], dt.bfloat16, kind="ExternalInput")
    wsb_d = nc.dram_tensor("wsb", [128, 384], dt.bfloat16, kind="ExternalInput")
    bia_d = nc.dram_tensor("bia", [1, 512], dt.bfloat16, kind="ExternalInput")
    rwt_d = nc.dram_tensor("rwt", [1, RPC], dt.bfloat16, kind="ExternalInput")
    xg_d = nc.dram_tensor("xg", [128, T_A * 128], dt.bfloat16, kind="ExternalInput")
    pta_d = nc.dram_tensor("pta", [128, T_A * 128], dt.bfloat16, kind="ExternalInput")
    ptb_d = nc.dram_tensor("ptb", [128, T_B * 128], dt.bfloat16, kind="ExternalInput")
    gix_d = nc.dram_tensor("gix", [128, T_B * 8], dt.int16, kind="ExternalInput")
    iot_d = nc.dram_tensor("iot", [128, 128], dt.bfloat16, kind="ExternalInput")
    offa_d = nc.dram_tensor("offa", [128, T_A], dt.float32, kind="ExternalInput")
    vala_d = nc.dram_tensor("vala", [128, T_A], dt.float32, kind="ExternalInput")
    o0_d = nc.dram_tensor("o0T", [128, NW * 128], dt.bfloat16, kind="ExternalOutput")
    o1_d = nc.dram_tensor("o1T", [128, NW * 128], dt.bfloat16, kind="ExternalOutput")
    o2_d = nc.dram_tensor("o2", [128, NW * 128], dt.bfloat16, kind="ExternalOutput")

    qn = [0]

    with tile.TileContext(nc) as tc:
        with (
            tc.tile_pool(name="const", bufs=1) as constp,
            tc.tile_pool(name="work", bufs=3) as workp,
            tc.tile_pool(name="big", bufs=1) as bigp,
            tc.tile_pool(name="gw", bufs=2) as gwp,
            tc.tile_pool(name="psS", bufs=2, space="PSUM") as psSp,
            tc.tile_pool(name="psc", bufs=4, space="PSUM") as pscp,
            tc.tile_pool(name="ps2", bufs=2, space="PSUM") as ps2p,
            tc.tile_pool(name="dram", bufs=1, space="DRAM") as dramp,
        ):
            xT = constp.tile([128, RPC], dt.bfloat16)
            wsb = constp.tile([128, 384], dt.bfloat16)
            nc.sync.dma_start(wsb[:], wsb_d[:])
            bia = constp.tile([1, 512], dt.bfloat16)
            nc.sync.dma_start(bia[:], bia_d[:])
            rwt = constp.tile([1, RPC], dt.bfloat16)
            nc.sync.dma_start(rwt[:], rwt_d[:])
            gix = constp.tile([128, T_B * 8], dt.int16)
            iot = constp.tile([128, 128], dt.bfloat16)
            nc.sync.dma_start(iot[:], iot_d[:])
            offa = constp.tile([128, T_A], dt.float32)
            nc.sync.dma_start(offa[:], offa_d[:])
            vala = constp.tile([128, T_A], dt.float32)
            nc.sync.dma_start(vala[:], vala_d[:])

            g_sh = [dramp.tile([NWH * 128, 128], dt.bfloat16, name=f"gsh{h}")
                    for h in (0, 1)]
            g_fl = [dramp.tile([HROWS, 128], dt.bfloat16,
                               addr_space="Shared", name=f"gfl{h}")
                    for h in (0, 1)]

            ssb = bigp.tile([128, NW * 128], dt.bfloat16, name="ssb")
            o0sb = bigp.tile([128, NW * 128], dt.bfloat16, name="o0sb")
            o1sb = bigp.tile([128, NW * 128], dt.bfloat16, name="o1sb")

            def emit_ag(h):
                nc.gpsimd.collective_compute(
                    "AllGather", mybir.AluOpType.bypass,
                    replica_groups=[list(range(CORES))],
                    ins=[g_sh[h][:].opt()],
                    outs=[g_fl[h][:].opt()])

            # ---- pass A phase 1: S = A@x scatter; g = S W2 + rw(x)b2 ----
            with (
                tc.tile_pool(name="xg", bufs=2) as xgp,
                tc.tile_pool(name="pta", bufs=2) as ptap,
            ):
                for (w0, w1) in (SG_A if STAGES >= 1 else []):
                    c0, c1 = int(cstartA[w0]), int(cstartA[w1])
                    xg = xgp.tile([128, (c1 - c0) * 128], dt.bfloat16, tag="xg")
                    nc.sync.dma_start(xg[:], xg_d[:, c0 * 128:c1 * 128])
                    pta = ptap.tile([128, (c1 - c0) * 128], dt.bfloat16, tag="pta")
                    cut = c0 + max(1, int(round((c1 - c0) * (1.0 - PTDVE))))
                    cut = min(cut, c1)
                    nc.sync.dma_start(pta[:, 0:(cut - c0) * 128],
                                      pta_d[:, c0 * 128:cut * 128])
                    for cg in range(cut, c1):
                        nc.vector.tensor_scalar(
                            pta[:, (cg - c0) * 128:(cg - c0 + 1) * 128],
                            iot[:], offa[:, cg:cg + 1], vala[:, cg:cg + 1],
                            mybir.AluOpType.is_equal, mybir.AluOpType.mult)
                    h = int(w0 >= NWH)
                    gsg = gwp.tile([128, w1 - w0, 128], dt.bfloat16, tag="gsg")

                    def emit_g(w):
                        # g = S W2 + rw (x) b2  (row-major directly)
                        ws = slice(w * 128, (w + 1) * 128)
                        psG = pscp.tile([128, 128], dt.float32, tag="psc")
                        nc.tensor.matmul(psG[:], ssb[:, ws], wsb[:, 256:384],
                                         start=True, stop=False)
                        nc.tensor.matmul(psG[:], rwt[0:1, ws],
                                         bia[0:1, 256:384], start=False, stop=True)
                        nc.vector.tensor_copy(gsg[:, w - w0, :], psG[:])

                    for w in range(w0, w1):
                        psS = psSp.tile([128, 128], dt.float32, tag="psS")
                        nb = int(BwA[w])
                        cw = int(cstartA[w]) - c0
                        for k in range(nb):
                            sl = slice((cw + k) * 128, (cw + k + 1) * 128)
                            nc.tensor.matmul(psS[:], xg[:, sl], pta[:, sl],
                                             start=(k == 0), stop=(k == nb - 1))
                        ws = slice(w * 128, (w + 1) * 128)
                        nc.scalar.copy(ssb[:, ws], psS[:])
                        if w > w0:
                            emit_g(w - 1)
                    emit_g(w1 - 1)
                    wl = w0 - NWH * h
                    gv = g_sh[h][wl * 128:(wl * 128 + (w1 - w0) * 128), :]
                    gv = gv.rearrange("(g a) c -> a g c", a=128)
                    nc.scalar.dma_start(gv, gsg[:])
                    if w1 == NWH and STAGES >= 2:
                        emit_ag(0)
            if STAGES >= 2:
                emit_ag(1)
            nc.sync.dma_start(xT[:], xT_d[:])
            nc.sync.dma_start(gix[:], gix_d[:])

            # ---- pass A phase 2: y1T / h0T from stored S^T (overlaps AG/B) --
            if STAGES >= 1:
                for w in range(NW):
                    ws = slice(w * 128, (w + 1) * 128)
                    ps2 = ps2p.tile([128, 256], dt.float32, tag="ps2")
                    nc.tensor.matmul(ps2[:, 0:128], wsb[:, 128:256],
                                     ssb[:, ws], start=True, stop=False)
                    nc.tensor.matmul(ps2[:, 0:128], bia[0:1, 128:256],
                                     rwt[0:1, ws], start=False, stop=True)
                    nc.tensor.matmul(ps2[:, 128:256], wsb[:, 0:128],
                                     xT[:, ws], start=True, stop=False)
                    nc.tensor.matmul(ps2[:, 128:256], bia[0:1, 0:128],
                                     bia[0:1, 384:512], start=False, stop=True)
                    nc.vector.tensor_copy(o1sb[:, ws], ps2[:, 0:128])
                    nc.scalar.copy(o0sb[:, ws], ps2[:, 128:256])

            # ---------------- pass B ----------------
            if STAGES >= 3:
                H0A = 4          # h=0 gathers issued this many groups ahead
                H1A = 2          # h=1 gathers issued this many groups ahead
                with (
                    tc.tile_pool(name="gath", bufs=H0A + 1) as gathp,
                    tc.tile_pool(name="ptb", bufs=3) as ptbp,
                ):
                    pend = {}

                    def issue_h(gi, h):
                        call = callsB[gi * 2 + h]
                        nch = call["nch"]
                        cs = call["c0"]
                        gt = gathp.tile([128, nch, 128], dt.bfloat16,
                                        tag=f"g{h}", name=f"gt{h}",
                                        bufs=(H0A + 1) if h == 0 else (H1A + 1))
                        nc.gpsimd.dma_gather(
                            gt[:], g_fl[h][:, :],
                            gix[:, cs * 8:(cs + nch) * 8],
                            num_idxs=nch * 128, num_idxs_reg=nch * 128,
                            elem_size=128, elem_step=128,
                            single_packet=False, queue_num=qn[0] % NQ)
                        qn[0] += 1
                        pend.setdefault(gi, {})[h] = (gt, cs)

                    def issue_pt(gi):
                        c0 = callsB[gi * 2]["c0"]
                        c1 = callsB[gi * 2 + 1]["c0"] + callsB[gi * 2 + 1]["nch"]
                        ptb = ptbp.tile([128, (c1 - c0) * 128], dt.bfloat16,
                                        tag="ptb")
                        nc.sync.dma_start(ptb[:], ptb_d[:, c0 * 128:c1 * 128])
                        pend[gi]["ptb"] = (ptb, c0)

                    def process(gi):
                        w0, w1 = SG_B[gi]
                        ent = pend.pop(gi)
                        ptb, c0 = ent["ptb"]
                        o2sg = gwp.tile([128, (w1 - w0) * 128], dt.bfloat16,
                                        tag="o2sg")
                        for w in range(w0, w1):
                            ktot = int(BwB[w, 0] + BwB[w, 1])
                            psY = pscp.tile([128, 128], dt.float32, tag="psc")
                            k = 0
                            for h in (0, 1):
                                gt, cs = ent[h]
                                for bch in range(int(BwB[w, h])):
                                    cg = int(cstartB[w, h]) + bch
                                    nc.tensor.matmul(
                                        psY[:],
                                        ptb[:, (cg - c0) * 128:(cg - c0 + 1) * 128],
                                        gt[:, cg - cs, :],
                                        start=(k == 0), stop=(k == ktot - 1))
                                    k += 1
                            wsl = slice((w - w0) * 128, (w - w0 + 1) * 128)
                            if w % 2 == 0:
                                nc.vector.tensor_copy(o2sg[:, wsl], psY[:])
                            else:
                                nc.scalar.copy(o2sg[:, wsl], psY[:])
                        nc.scalar.dma_start(
                            o2_d[:, w0 * 128:w1 * 128], o2sg[:])

                    nG = len(SG_B)
                    for step in range(nG + H0A):
                        if step < nG:
                            issue_h(step, 0)
                        g1 = step - (H0A - H1A)
                        if 0 <= g1 < nG:
                            issue_h(g1, 1)
                            issue_pt(g1)
                        pk = step - H0A
                        if 0 <= pk < nG:
                            process(pk)
            if STAGES >= 1:
                nc.sync.dma_start(o1_d[:], o1sb[:])
                nc.sync.dma_start(o0_d[:], o0sb[:])

    nc.compile()
    return nc


def _prepare_inputs(x, W, b, plan):
    relabel = plan["relabel"]
    xpad = np.zeros((NP, C), np.float32)
    xpad[relabel[:N]] = x
    xbf = xpad.astype(ml_dtypes.bfloat16)
    xT_all = np.ascontiguousarray(xbf.T)           # [128, NP]

    wsb = np.concatenate([W[0], W[1], W[2]], axis=1).astype(ml_dtypes.bfloat16)
    bia = np.zeros((1, 512), np.float32)
    bia[0, 0:384] = np.concatenate([b[0], b[1], b[2]])
    bia[0, 384:512] = 1.0
    bia = bia.astype(ml_dtypes.bfloat16)

    rw = plan["rowsum"].astype(ml_dtypes.bfloat16)  # [NP]

    T_A = plan["T_A"]
    colA = plan["colA"]                             # [CORES, T_A*128]
    in_maps = []
    for c in range(CORES):
        # pre-gathered x rows, laid out [slot%128 partition, chunk, C]
        xg = xbf[colA[c]]                           # [T_A*128, 128]
        xg = xg.reshape(T_A, 128, 128).transpose(1, 0, 2)
        xg = np.ascontiguousarray(xg.reshape(128, T_A * 128))
        in_maps.append({
            "xT": np.ascontiguousarray(xT_all[:, c * RPC:(c + 1) * RPC]),
            "wsb": wsb,
            "bia": bia,
            "rwt": np.ascontiguousarray(rw[c * RPC:(c + 1) * RPC]).reshape(1, RPC),
            "xg": xg,
            "pta": plan["ptA"][c],
            "ptb": plan["ptB"][c],
            "gix": plan["gix"][c],
            "iot": np.broadcast_to(np.arange(128, dtype=np.float32), (128, 128)
                                   ).astype(ml_dtypes.bfloat16),
            "offa": plan["offa"][c],
            "vala": plan["vala"][c],
        })
    return in_maps


def kernel(x, W, b, edge_val, edge_row, edge_col):
    x = np.asarray(x, np.float32)
    W = np.asarray(W, np.float32)
    b = np.asarray(b, np.float32)
    edge_val = np.asarray(edge_val, np.float32)
    edge_row = np.asarray(edge_row, np.int32)
    edge_col = np.asarray(edge_col, np.int32)

    from concourse.bass_utils import run_bass_kernel_spmd

    key = hash((edge_row.tobytes(), edge_col.tobytes(), edge_val.tobytes()))
    if key not in _CACHE:
        plan = _build_plan(edge_row, edge_col, edge_val)
        nc = _build_program(plan)
        _CACHE[key] = (plan, nc)
    plan, nc = _CACHE[key]

    in_maps = _prepare_inputs(x, W, b, plan)
    res = run_bass_kernel_spmd(nc, in_maps, core_ids=list(range(CORES)),
                               trace=TRACE)
    kernel.last_results = res
    parts = []
    for c in range(CORES):
        r = res.results[c]
        # o0T/o1T: [128 och, NW*128 (w,row)] ; o2: [128 row, NW*128 (w,och)]
        h0 = np.asarray(r["o0T"], dtype=np.float32).reshape(128, NW, 128)
        y1 = np.asarray(r["o1T"], dtype=np.float32).reshape(128, NW, 128)
        y2 = np.asarray(r["o2"], dtype=np.float32).reshape(128, NW, 128)
        h0 = h0.transpose(1, 2, 0).reshape(RPC, 128)
        y1 = y1.transpose(1, 2, 0).reshape(RPC, 128)
        y2 = y2.transpose(1, 0, 2).reshape(RPC, 128)
        parts.append(np.concatenate([h0, y1, y2], axis=1))
    full = np.concatenate(parts, axis=0)
    return np.ascontiguousarray(full[plan["relabel"][:N]])


kernel.last_results = None


if __name__ == "__main__":
    rng = np.random.default_rng(0)
    x = rng.standard_normal((N, C), dtype=np.float32)
    W = rng.standard_normal((3, C, C), dtype=np.float32) / np.sqrt(C)
    b = rng.standard_normal((3, C), dtype=np.float32) * 0.01
    ev = rng.random(E, dtype=np.float32)
    er = rng.integers(0, N, E, dtype=np.int32)
    ec = rng.integers(0, N, E, dtype=np.int32)
    out = kernel(x=x, W=W, b=b, edge_val=ev, edge_row=er, edge_col=ec)
    print(out.shape, out.dtype)
